# revision 29
# baseline (speedup 1.0000x reference)
"""Distributed Trainium2 Bass kernel for nn_Attention_14044543058524.

Reference computation (per problem):
    transformed = einsum('dbh,doh->dbo', feats, weights)      # per-d linear
    unit        = transformed / ||transformed||_rows           # L2 row-normalize
    scores      = einsum('ibh,jbh->ij', unit, unit) / B        # [D, D]
    attn        = softmax(scores, axis=1)
    out         = einsum('dg,gbh->dbh', attn, feats)

Strategy: data-parallel over B across 8 NeuronCores.  Each core:
  pass 1: t = f @ W^T (fp16 TensorE, PSUM f32); pair dot products
          dot_ij[b] = sum_o t_i[b,o] t_j[b,o] fused on DVE
          (scalar_tensor_tensor + accumulate) and ScalarE (square +
          accumulate); per-row normalization applied on tiny [128, 10, NB]
          tensors, partition-reduced with a ones-matmul.
  comm:   64-byte AllReduce of the 16 gram entries.
  tail:   softmax of the 4x4 scores on one partition (exp / reduce /
          reciprocal / tensor_scalar), broadcast to partitions, scaled
          identity matrices attn[d,g] * I.
  pass 2: out_d = sum_g attn[d,g] f_g via PSUM-accumulated identity matmuls.

Host pre-transposes feats to [D, H, B_loc] fp16 so the h-contraction axis is
the SBUF partition axis on-chip (no on-chip transposes at all).
"""

import numpy as np

D, B, H = 4, 16384, 1024
NCORES = 8
BL_FULL = B // NCORES  # 2048

# self pairs first (their dots are the squared row norms)
PAIRS = [(0, 0), (1, 1), (2, 2), (3, 3),
         (0, 1), (0, 2), (0, 3), (1, 2), (1, 3), (2, 3)]
NPAIR = len(PAIRS)
# cell (i, j) of the 4x4 score matrix -> unique pair index
CELL2PAIR = [PAIRS.index((min(i, j), max(i, j)))
             for i in range(4) for j in range(4)]

_CACHE = {}


def _build_nc(bl):
    """Build + compile the SPMD Bass graph for per-core batch size `bl`."""
    from concourse import bass, bacc, tile, masks

    mybir = bass.mybir
    f16 = mybir.dt.float16
    f32 = mybir.dt.float32
    f8 = mybir.dt.float8e4
    MULT = mybir.AluOpType.mult
    ADD = mybir.AluOpType.add
    AF = mybir.ActivationFunctionType

    nb = bl // 128          # b-tiles of 128 per core
    nhc = H // 128          # 8 h-chunks
    fq_w = min(512, bl)     # ft1 quarter width (b columns per resident tile)
    nfq = bl // fq_w
    bb_w = min(1024, bl)    # pass-2 output tile width (2 PSUM banks)
    nbb = bl // bb_w
    mm_w = min(512, bb_w)   # pass-2 matmul moving width
    nmm = bb_w // mm_w

    nc = bacc.Bacc("TRN2", target_bir_lowering=False, debug=False,
                   num_devices=NCORES)

    ft_d = nc.dram_tensor("ft", [D, H, bl], f16, kind="ExternalInput")
    ft8_d = nc.dram_tensor("ft8", [D, H, bl], f8, kind="ExternalInput")
    wt8_d = nc.dram_tensor("wt8", [D, H, H], f8, kind="ExternalInput")
    out_d = nc.dram_tensor("out", [D, H, bl], f16, kind="ExternalOutput")

    # expand matrix: unique-pair index -> 4x4 cell (0/1), used to spread the
    # 10 unique gram entries onto 16 partitions with one tiny matmul
    expand_np = np.zeros((NPAIR, 16), np.float32)
    for c, k in enumerate(CELL2PAIR):
        expand_np[k, c] = 1.0
    expand_dram = nc.inline_tensor(expand_np, "expandmask")

    with tile.TileContext(nc) as tc:
        with (
            tc.tile_pool(name="const", bufs=1) as constp,
            tc.tile_pool(name="wt", bufs=1) as wtp,
            tc.tile_pool(name="ft1", bufs=2) as ft1p,
            tc.tile_pool(name="tt", bufs=3) as ttp,
            tc.tile_pool(name="work", bufs=3) as workp,
            tc.tile_pool(name="small", bufs=1) as smallp,
            tc.tile_pool(name="ident", bufs=1) as identp,
            tc.tile_pool(name="ft2", bufs=8) as ft2p,
            tc.tile_pool(name="ost", bufs=6) as ostp,
            tc.tile_pool(name="psum", bufs=3, space="PSUM") as psump,
            tc.tile_pool(name="psmall", bufs=2, space="PSUM") as psmallp,
            tc.tile_pool(name="dram", bufs=1, space="DRAM") as dramp,
        ):
            # ---- constants + ACT table warm-up -----------------------------
            ones = constp.tile([128, 1], f32, tag="ones")
            nc.vector.memset(ones[:], 1.0)
            warm = constp.tile([1, 1], f32, tag="warm")
            nc.vector.memset(warm[:], 1.0)
            # load the Sqrt and Exp spline tables off the critical path
            nc.scalar.activation(warm[:], warm[:], AF.Sqrt)
            nc.scalar.activation(warm[:], warm[:], AF.Exp)
            ident_base = constp.tile([128, 128], f16, tag="identity")
            masks.make_identity(nc, ident_base[:])
            exm = constp.tile([NPAIR, 16], f32, tag="exm")
            nc.sync.dma_start(exm[:], expand_dram[:])

            dots = smallp.tile([128, NPAIR, nb], f32, tag="dots")
            ar_split = max(1, nb // 2)
            arin0 = dramp.tile([16, 1], f32)
            arin1 = dramp.tile([16, 1], f32)
            arout0 = dramp.tile([NCORES, 16], f32)
            arout1 = dramp.tile([NCORES, 16], f32)
            arins = [arin0, arin1]
            arouts = [arout0, arout1]

            ftap = ft_d[:]  # [D, H, bl]

            # ---- weights resident (interleaved with the first ft1 loads so
            # ---- d=0 can start its matmuls as early as possible) -----------
            nhcp = nhc // 2
            wt_sb = []
            ft1_tiles = {}
            ft8ap = ft8_d[:]
            for d in range(D):
                per_d = []
                for hcp in range(nhcp):
                    t = wtp.tile([128, 2, H], f8, tag=f"wt_{d}_{hcp}")
                    nc.sync.dma_start(
                        t[:],
                        wt8_d[d, hcp * 256:(hcp + 1) * 256, :].rearrange(
                            "(i p) o -> p i o", p=128))
                    per_d.append(t)
                wt_sb.append(per_d)
                ftile = ft1p.tile([128, nhc, fq_w], f8, tag=f"ft1_{d}")
                src0 = ft8ap[d].rearrange("(hc p) b -> p hc b", p=128)
                nc.sync.dma_start(ftile[:], src0[:, :, 0:fq_w])
                ft1_tiles[(d, 0)] = ftile

            # ---- pass 1: t = f @ W^T, pair dots ----------------------------
            bt_per_q = fq_w // 128
            for fq in range(nfq):
                # resident stationary tiles for this quarter: [h128][hc][b fq_w]
                ft1 = []
                for d in range(D):
                    if (d, fq) in ft1_tiles:
                        ft1.append(ft1_tiles[(d, fq)])
                        continue
                    ftile = ft1p.tile([128, nhc, fq_w], f8, tag=f"ft1_{d}")
                    src = ft8ap[d].rearrange("(hc p) b -> p hc b", p=128)
                    nc.sync.dma_start(
                        ftile[:], src[:, :, fq * fq_w:(fq + 1) * fq_w])
                    ft1.append(ftile)

                for btq in range(bt_per_q):
                    bt = fq * bt_per_q + btq
                    b0 = btq * 128
                    t_sb = []
                    for d in range(D):
                        t_t = ttp.tile([128, H], f16, tag=f"t_{d}")
                        ps = psump.tile([128, H], f32, tag="pm")
                        for hcp in range(nhcp):
                            st = ft1[d][:, 2 * hcp:2 * hcp + 2, b0:b0 + 128]
                            nc.tensor.matmul(
                                ps[:, 0:512], lhsT=st,
                                rhs=wt_sb[d][hcp][:, :, 0:512],
                                start=(hcp == 0), stop=(hcp == nhcp - 1),
                                perf_mode=mybir.MatmulPerfMode.DoubleRow,
                                skip_group_check=True)
                            nc.tensor.matmul(
                                ps[:, 512:1024], lhsT=st,
                                rhs=wt_sb[d][hcp][:, :, 512:1024],
                                start=(hcp == 0), stop=(hcp == nhcp - 1),
                                perf_mode=mybir.MatmulPerfMode.DoubleRow,
                                skip_group_check=True)
                        nc.scalar.copy(t_t[:], ps[:])
                        t_sb.append(t_t)

                    # pair dots: 2 self pairs on ScalarE (square+accum), the
                    # rest fused multiply+accumulate on VectorE.
                    for k, (i, j) in enumerate(PAIRS):
                        prod = workp.tile([128, H], f16, tag="prod")
                        if i == j:
                            nc.scalar.activation(
                                prod[:], t_sb[i][:], AF.Square,
                                accum_out=dots[:, k, bt:bt + 1])
                        else:
                            nc.vector.scalar_tensor_tensor(
                                out=prod[:],
                                in0=t_sb[i][:],
                                scalar=1.0,
                                in1=t_sb[j][:],
                                op0=MULT,
                                op1=MULT,
                                accum_out=dots[:, k, bt:bt + 1],
                            )

                    # after btile ar_split-1: launch the first partial
                    # AllGather so its ~45us latency hides under the rest of
                    # pass 1; the second partial goes right after the last
                    # tile.
                    if bt in (ar_split - 1, nb - 1):
                        half = 0 if bt == ar_split - 1 else 1
                        lo = 0 if half == 0 else ar_split
                        hi = ar_split if half == 0 else nb
                        w = hi - lo
                        sqh = smallp.tile([128, 4, w], f32, tag=f"sq_{half}")
                        nc.scalar.sqrt(sqh[:], dots[:, 0:4, lo:hi])
                        invh = smallp.tile([128, 4, w], f32, tag=f"inv_{half}")
                        nc.vector.reciprocal(invh[:], sqh[:])
                        qh = smallp.tile([128, NPAIR, w], f32, tag=f"q_{half}")
                        for k, (i, j) in enumerate(PAIRS):
                            nc.vector.tensor_tensor(
                                out=qh[:, k, :], in0=dots[:, k, lo:hi],
                                in1=invh[:, i, :], op=MULT)
                            nc.vector.tensor_tensor(
                                out=qh[:, k, :], in0=qh[:, k, :],
                                in1=invh[:, j, :], op=MULT)
                        gsh = smallp.tile([128, NPAIR], f32, tag=f"gs_{half}")
                        nc.vector.tensor_reduce(
                            out=gsh[:], in_=qh[:], axis=mybir.AxisListType.X,
                            op=ADD)
                        z10p = psmallp.tile([NPAIR, 1], f32, tag="ps_small")
                        nc.tensor.matmul(z10p[:], lhsT=gsh[:], rhs=ones[:],
                                         start=True, stop=True)
                        z10 = smallp.tile([NPAIR, 1], f32, tag=f"z10_{half}")
                        nc.scalar.copy(z10[:], z10p[:])
                        z16p = psmallp.tile([16, 1], f32, tag="ps_small")
                        nc.tensor.matmul(z16p[:], lhsT=exm[:], rhs=z10[:],
                                         start=True, stop=True)
                        z16 = smallp.tile([16, 1], f32, tag=f"z16_{half}")
                        nc.scalar.copy(z16[:], z16p[:])
                        nc.sync.dma_start(arins[half][:], z16[:])
                        nc.gpsimd.collective_compute(
                            "AllGather",
                            mybir.AluOpType.bypass,
                            ins=[arins[half].opt()],
                            outs=[arouts[half].opt()],
                            replica_groups=[list(range(NCORES))],
                        )


            # ---- gather the two partial gram sums --------------------------
            ag0 = smallp.tile([NCORES, 16], f32, tag="ag0")
            nc.sync.dma_start(ag0[:], arouts[0][:])
            ag1 = smallp.tile([NCORES, 16], f32, tag="ag1")
            nc.sync.dma_start(ag1[:], arouts[1][:])
            sp = psmallp.tile([1, 16], f32, tag="ps_small")
            nc.tensor.matmul(sp[:], lhsT=ones[0:NCORES, :], rhs=ag0[:],
                             start=True, stop=False, skip_group_check=True)
            nc.tensor.matmul(sp[:], lhsT=ones[0:NCORES, :], rhs=ag1[:],
                             start=False, stop=True, skip_group_check=True)
            srow_t = smallp.tile([1, 16], f32, tag="srow")
            nc.scalar.copy(srow_t[:], sp[:])
            srow = srow_t[:].rearrange("o (a b) -> o a b", a=4)
            erow = smallp.tile([1, 4, 4], f32, tag="erow")
            # scores = gram / B; scores_ii == 1 so exp never overflows
            nc.scalar.activation(erow[:], srow, AF.Exp,
                                 scale=1.0 / (bl * NCORES))
            rsum = smallp.tile([1, 4], f32, tag="rsum")
            nc.vector.tensor_reduce(out=rsum[:], in_=erow[:],
                                    axis=mybir.AxisListType.X, op=ADD)
            rinv = smallp.tile([1, 4], f32, tag="rinv")
            nc.vector.reciprocal(rinv[:], rsum[:])
            attnrow = smallp.tile([1, 16], f32, tag="attnrow")
            arview = attnrow[:].rearrange("o (a b) -> o a b", a=4)
            for r in range(4):
                nc.vector.tensor_scalar(
                    out=arview[:, r, :], in0=erow[:, r, :],
                    scalar1=rinv[:, r:r + 1], scalar2=None, op0=MULT)
            attnb = smallp.tile([128, 16], f32, tag="attnb")
            nc.gpsimd.partition_broadcast(attnb[:], attnrow[:])

            idents = []
            for k in range(16):
                idk = identp.tile([128, 128], f16, tag=f"id_{k}")
                nc.vector.tensor_scalar(
                    out=idk[:], in0=ident_base[:],
                    scalar1=attnb[:, k:k + 1], scalar2=None, op0=MULT)
                idents.append(idk)

            # ---- pass 2: out_d = sum_g attn[d,g] f_g -----------------------
            for hc in range(nhc):
                for bb in range(nbb):
                    fg = []
                    for g in range(D):
                        t = ft2p.tile([128, bb_w], f16, tag=f"ft2_{g}")
                        nc.sync.dma_start(
                            t[:],
                            ftap[g, hc * 128:(hc + 1) * 128,
                                 bb * bb_w:(bb + 1) * bb_w])
                        fg.append(t)
                    for d2 in range(D):
                        # balance pass 2 between TensorE (identity matmuls)
                        # and VectorE (4x tensor_scalar + 2x adds) -- both
                        # engines produce ~same tile rate, halving the span.
                        on_dve = (d2 == 3) or (d2 == 2 and hc % 2 == 1)
                        if on_dve:
                            acc = ostp.tile([128, bb_w], f16, tag="ost_dve")
                            tmp = workp.tile([128, bb_w], f16, tag="p2tmp")
                            nc.vector.tensor_scalar(
                                out=acc[:], in0=fg[0][:],
                                scalar1=attnb[:, d2 * 4:d2 * 4 + 1],
                                scalar2=None, op0=MULT)
                            for g in range(1, D):
                                nc.vector.tensor_scalar(
                                    out=tmp[:], in0=fg[g][:],
                                    scalar1=attnb[:, d2 * 4 + g:d2 * 4 + g + 1],
                                    scalar2=None, op0=MULT)
                                nc.vector.tensor_tensor(
                                    out=acc[:], in0=acc[:], in1=tmp[:], op=ADD)
                            nc.sync.dma_start(
                                out_d[d2, hc * 128:(hc + 1) * 128,
                                      bb * bb_w:(bb + 1) * bb_w], acc[:])
                            continue
                        po = psump.tile([128, bb_w], f32, tag="pm")
                        for m in range(nmm):
                            sl = slice(m * mm_w, (m + 1) * mm_w)
                            for g in range(D):
                                nc.tensor.matmul(
                                    po[:, sl], lhsT=idents[d2 * 4 + g][:],
                                    rhs=fg[g][:, sl],
                                    start=(g == 0), stop=(g == D - 1),
                                    skip_group_check=True)
                        os_t = ostp.tile([128, bb_w], f16, tag="ost")
                        nc.scalar.copy(os_t[:], po[:])
                        nc.sync.dma_start(
                            out_d[d2, hc * 128:(hc + 1) * 128,
                                  bb * bb_w:(bb + 1) * bb_w], os_t[:])

    nc.compile()
    return nc


def _get_nc(bl):
    if bl not in _CACHE:
        _CACHE[bl] = _build_nc(bl)
    return _CACHE[bl]


def _host_prep(feats, weights, bl):
    """Shard + transpose + cast inputs for each core."""
    import ml_dtypes
    ncores = feats.shape[1] // bl
    wtT = np.transpose(weights, (0, 2, 1))                    # [D, H_in, H_out]
    wt8 = np.ascontiguousarray((wtT * 16.0).astype(ml_dtypes.float8_e4m3))
    ftT = np.transpose(feats, (0, 2, 1))                      # [D, H, B]
    ftT16 = ftT.astype(np.float16)
    ftT8 = ftT.astype(ml_dtypes.float8_e4m3)
    in_maps = []
    for c in range(ncores):
        sl = slice(c * bl, (c + 1) * bl)
        in_maps.append({
            "ft": np.ascontiguousarray(ftT16[:, :, sl]),
            "ft8": np.ascontiguousarray(ftT8[:, :, sl]),
            "wt8": wt8,
        })
    return in_maps


def _assemble(results, bl):
    ncores = len(results)
    out = np.empty((D, ncores * bl, H), dtype=np.float32)
    for c, res in enumerate(results):
        # res["out"]: [D, H, bl] fp16
        out[:, c * bl:(c + 1) * bl, :] = np.transpose(
            res["out"].astype(np.float32), (0, 2, 1))
    return out


def run(feats, weights, trace=False, bl=BL_FULL, **spmd_kwargs):
    from concourse import bass_utils
    nc = _get_nc(bl)
    in_maps = _host_prep(np.asarray(feats), np.asarray(weights), bl)
    res = bass_utils.run_bass_kernel_spmd(
        nc, in_maps, core_ids=list(range(NCORES)), trace=trace, **spmd_kwargs)
    return _assemble(res.results, bl), res


def kernel(feats, weights):
    out, _ = run(np.asarray(feats), np.asarray(weights))
    return out


# revision 30
# speedup vs baseline: 1.0117x; 1.0117x over previous
"""Distributed Trainium2 Bass kernel for nn_Attention_14044543058524.

Reference computation (per problem):
    transformed = einsum('dbh,doh->dbo', feats, weights)      # per-d linear
    unit        = transformed / ||transformed||_rows           # L2 row-normalize
    scores      = einsum('ibh,jbh->ij', unit, unit) / B        # [D, D]
    attn        = softmax(scores, axis=1)
    out         = einsum('dg,gbh->dbh', attn, feats)

Strategy: data-parallel over B across 8 NeuronCores.  Each core:
  pass 1: t = f @ W^T (fp16 TensorE, PSUM f32); pair dot products
          dot_ij[b] = sum_o t_i[b,o] t_j[b,o] fused on DVE
          (scalar_tensor_tensor + accumulate) and ScalarE (square +
          accumulate); per-row normalization applied on tiny [128, 10, NB]
          tensors, partition-reduced with a ones-matmul.
  comm:   64-byte AllReduce of the 16 gram entries.
  tail:   softmax of the 4x4 scores on one partition (exp / reduce /
          reciprocal / tensor_scalar), broadcast to partitions, scaled
          identity matrices attn[d,g] * I.
  pass 2: out_d = sum_g attn[d,g] f_g via PSUM-accumulated identity matmuls.

Host pre-transposes feats to [D, H, B_loc] fp16 so the h-contraction axis is
the SBUF partition axis on-chip (no on-chip transposes at all).
"""

import numpy as np

D, B, H = 4, 16384, 1024
NCORES = 8
BL_FULL = B // NCORES  # 2048

# self pairs first (their dots are the squared row norms)
PAIRS = [(0, 0), (1, 1), (2, 2), (3, 3),
         (0, 1), (0, 2), (0, 3), (1, 2), (1, 3), (2, 3)]
NPAIR = len(PAIRS)
# cell (i, j) of the 4x4 score matrix -> unique pair index
CELL2PAIR = [PAIRS.index((min(i, j), max(i, j)))
             for i in range(4) for j in range(4)]

_CACHE = {}


def _build_nc(bl):
    """Build + compile the SPMD Bass graph for per-core batch size `bl`."""
    from concourse import bass, bacc, tile, masks

    mybir = bass.mybir
    f16 = mybir.dt.float16
    f32 = mybir.dt.float32
    f8 = mybir.dt.float8e4
    MULT = mybir.AluOpType.mult
    ADD = mybir.AluOpType.add
    AF = mybir.ActivationFunctionType

    nb = bl // 128          # b-tiles of 128 per core
    nhc = H // 128          # 8 h-chunks
    fq_w = min(512, bl)     # ft1 quarter width (b columns per resident tile)
    nfq = bl // fq_w
    bb_w = min(1024, bl)    # pass-2 output tile width (2 PSUM banks)
    nbb = bl // bb_w
    mm_w = min(512, bb_w)   # pass-2 matmul moving width
    nmm = bb_w // mm_w

    nc = bacc.Bacc("TRN2", target_bir_lowering=False, debug=False,
                   num_devices=NCORES)

    ft_d = nc.dram_tensor("ft", [D, H, bl], f16, kind="ExternalInput")
    ft8_d = nc.dram_tensor("ft8", [D, H, bl], f8, kind="ExternalInput")
    wt8_d = nc.dram_tensor("wt8", [D, H, H], f8, kind="ExternalInput")
    out_d = nc.dram_tensor("out", [D, H, bl], f16, kind="ExternalOutput")

    # expand matrix: unique-pair index -> 4x4 cell (0/1), used to spread the
    # 10 unique gram entries onto 16 partitions with one tiny matmul
    expand_np = np.zeros((NPAIR, 16), np.float32)
    for c, k in enumerate(CELL2PAIR):
        expand_np[k, c] = 1.0
    expand_dram = nc.inline_tensor(expand_np, "expandmask")

    with tile.TileContext(nc) as tc:
        with (
            tc.tile_pool(name="const", bufs=1) as constp,
            tc.tile_pool(name="wt", bufs=1) as wtp,
            tc.tile_pool(name="ft1", bufs=2) as ft1p,
            tc.tile_pool(name="tt", bufs=3) as ttp,
            tc.tile_pool(name="work", bufs=3) as workp,
            tc.tile_pool(name="small", bufs=1) as smallp,
            tc.tile_pool(name="ident", bufs=1) as identp,
            tc.tile_pool(name="ft2", bufs=8) as ft2p,
            tc.tile_pool(name="ost", bufs=6) as ostp,
            tc.tile_pool(name="psum", bufs=3, space="PSUM") as psump,
            tc.tile_pool(name="psmall", bufs=2, space="PSUM") as psmallp,
            tc.tile_pool(name="dram", bufs=1, space="DRAM") as dramp,
        ):
            # ---- constants + ACT table warm-up -----------------------------
            ones = constp.tile([128, 1], f32, tag="ones")
            nc.vector.memset(ones[:], 1.0)
            warm = constp.tile([1, 1], f32, tag="warm")
            nc.vector.memset(warm[:], 1.0)
            # load the Sqrt and Exp spline tables off the critical path
            nc.scalar.activation(warm[:], warm[:], AF.Sqrt)
            nc.scalar.activation(warm[:], warm[:], AF.Exp)
            ident_base = constp.tile([128, 128], f16, tag="identity")
            masks.make_identity(nc, ident_base[:])
            exm = constp.tile([NPAIR, 16], f32, tag="exm")
            nc.sync.dma_start(exm[:], expand_dram[:])

            dots = smallp.tile([128, NPAIR, nb], f32, tag="dots")
            ar_split = max(1, (3 * nb) // 4)
            arin0 = dramp.tile([16, 1], f32)
            arin1 = dramp.tile([16, 1], f32)
            arout0 = dramp.tile([NCORES, 16], f32)
            arout1 = dramp.tile([NCORES, 16], f32)
            arins = [arin0, arin1]
            arouts = [arout0, arout1]

            ftap = ft_d[:]  # [D, H, bl]

            # ---- weights resident (interleaved with the first ft1 loads so
            # ---- d=0 can start its matmuls as early as possible) -----------
            nhcp = nhc // 2
            wt_sb = []
            ft1_tiles = {}
            ft8ap = ft8_d[:]
            for d in range(D):
                per_d = []
                for hcp in range(nhcp):
                    t = wtp.tile([128, 2, H], f8, tag=f"wt_{d}_{hcp}")
                    nc.sync.dma_start(
                        t[:],
                        wt8_d[d, hcp * 256:(hcp + 1) * 256, :].rearrange(
                            "(i p) o -> p i o", p=128))
                    per_d.append(t)
                wt_sb.append(per_d)
                ftile = ft1p.tile([128, nhc, fq_w], f8, tag=f"ft1_{d}")
                src0 = ft8ap[d].rearrange("(hc p) b -> p hc b", p=128)
                nc.sync.dma_start(ftile[:], src0[:, :, 0:fq_w])
                ft1_tiles[(d, 0)] = ftile

            # ---- pass 1: t = f @ W^T, pair dots ----------------------------
            bt_per_q = fq_w // 128
            for fq in range(nfq):
                # resident stationary tiles for this quarter: [h128][hc][b fq_w]
                ft1 = []
                for d in range(D):
                    if (d, fq) in ft1_tiles:
                        ft1.append(ft1_tiles[(d, fq)])
                        continue
                    ftile = ft1p.tile([128, nhc, fq_w], f8, tag=f"ft1_{d}")
                    src = ft8ap[d].rearrange("(hc p) b -> p hc b", p=128)
                    nc.sync.dma_start(
                        ftile[:], src[:, :, fq * fq_w:(fq + 1) * fq_w])
                    ft1.append(ftile)

                for btq in range(bt_per_q):
                    bt = fq * bt_per_q + btq
                    b0 = btq * 128
                    t_sb = []
                    for d in range(D):
                        t_t = ttp.tile([128, H], f16, tag=f"t_{d}")
                        ps = psump.tile([128, H], f32, tag="pm")
                        for hcp in range(nhcp):
                            st = ft1[d][:, 2 * hcp:2 * hcp + 2, b0:b0 + 128]
                            nc.tensor.matmul(
                                ps[:, 0:512], lhsT=st,
                                rhs=wt_sb[d][hcp][:, :, 0:512],
                                start=(hcp == 0), stop=(hcp == nhcp - 1),
                                perf_mode=mybir.MatmulPerfMode.DoubleRow,
                                skip_group_check=True)
                            nc.tensor.matmul(
                                ps[:, 512:1024], lhsT=st,
                                rhs=wt_sb[d][hcp][:, :, 512:1024],
                                start=(hcp == 0), stop=(hcp == nhcp - 1),
                                perf_mode=mybir.MatmulPerfMode.DoubleRow,
                                skip_group_check=True)
                        nc.scalar.copy(t_t[:], ps[:])
                        t_sb.append(t_t)

                    # pair dots: 2 self pairs on ScalarE (square+accum), the
                    # rest fused multiply+accumulate on VectorE.
                    for k, (i, j) in enumerate(PAIRS):
                        prod = workp.tile([128, H], f16, tag="prod")
                        if i == j:
                            nc.scalar.activation(
                                prod[:], t_sb[i][:], AF.Square,
                                accum_out=dots[:, k, bt:bt + 1])
                        else:
                            nc.vector.scalar_tensor_tensor(
                                out=prod[:],
                                in0=t_sb[i][:],
                                scalar=1.0,
                                in1=t_sb[j][:],
                                op0=MULT,
                                op1=MULT,
                                accum_out=dots[:, k, bt:bt + 1],
                            )

                    # after btile ar_split-1: launch the first partial
                    # AllGather so its ~45us latency hides under the rest of
                    # pass 1; the second partial goes right after the last
                    # tile.
                    if bt in (ar_split - 1, nb - 1):
                        half = 0 if bt == ar_split - 1 else 1
                        lo = 0 if half == 0 else ar_split
                        hi = ar_split if half == 0 else nb
                        w = hi - lo
                        sqh = smallp.tile([128, 4, w], f32, tag=f"sq_{half}")
                        nc.scalar.sqrt(sqh[:], dots[:, 0:4, lo:hi])
                        invh = smallp.tile([128, 4, w], f32, tag=f"inv_{half}")
                        nc.vector.reciprocal(invh[:], sqh[:])
                        qh = smallp.tile([128, NPAIR, w], f32, tag=f"q_{half}")
                        for k, (i, j) in enumerate(PAIRS):
                            nc.vector.tensor_tensor(
                                out=qh[:, k, :], in0=dots[:, k, lo:hi],
                                in1=invh[:, i, :], op=MULT)
                            nc.vector.tensor_tensor(
                                out=qh[:, k, :], in0=qh[:, k, :],
                                in1=invh[:, j, :], op=MULT)
                        gsh = smallp.tile([128, NPAIR], f32, tag=f"gs_{half}")
                        nc.vector.tensor_reduce(
                            out=gsh[:], in_=qh[:], axis=mybir.AxisListType.X,
                            op=ADD)
                        z10p = psmallp.tile([NPAIR, 1], f32, tag="ps_small")
                        nc.tensor.matmul(z10p[:], lhsT=gsh[:], rhs=ones[:],
                                         start=True, stop=True)
                        z10 = smallp.tile([NPAIR, 1], f32, tag=f"z10_{half}")
                        nc.scalar.copy(z10[:], z10p[:])
                        z16p = psmallp.tile([16, 1], f32, tag="ps_small")
                        nc.tensor.matmul(z16p[:], lhsT=exm[:], rhs=z10[:],
                                         start=True, stop=True)
                        z16 = smallp.tile([16, 1], f32, tag=f"z16_{half}")
                        nc.scalar.copy(z16[:], z16p[:])
                        nc.sync.dma_start(arins[half][:], z16[:])
                        nc.gpsimd.collective_compute(
                            "AllGather",
                            mybir.AluOpType.bypass,
                            ins=[arins[half].opt()],
                            outs=[arouts[half].opt()],
                            replica_groups=[list(range(NCORES))],
                        )


            # ---- gather the two partial gram sums --------------------------
            ag0 = smallp.tile([NCORES, 16], f32, tag="ag0")
            nc.sync.dma_start(ag0[:], arouts[0][:])
            ag1 = smallp.tile([NCORES, 16], f32, tag="ag1")
            nc.sync.dma_start(ag1[:], arouts[1][:])
            sp = psmallp.tile([1, 16], f32, tag="ps_small")
            nc.tensor.matmul(sp[:], lhsT=ones[0:NCORES, :], rhs=ag0[:],
                             start=True, stop=False, skip_group_check=True)
            nc.tensor.matmul(sp[:], lhsT=ones[0:NCORES, :], rhs=ag1[:],
                             start=False, stop=True, skip_group_check=True)
            srow_t = smallp.tile([1, 16], f32, tag="srow")
            nc.scalar.copy(srow_t[:], sp[:])
            srow = srow_t[:].rearrange("o (a b) -> o a b", a=4)
            erow = smallp.tile([1, 4, 4], f32, tag="erow")
            # scores = gram / B; scores_ii == 1 so exp never overflows
            nc.scalar.activation(erow[:], srow, AF.Exp,
                                 scale=1.0 / (bl * NCORES))
            rsum = smallp.tile([1, 4], f32, tag="rsum")
            nc.vector.tensor_reduce(out=rsum[:], in_=erow[:],
                                    axis=mybir.AxisListType.X, op=ADD)
            rinv = smallp.tile([1, 4], f32, tag="rinv")
            nc.vector.reciprocal(rinv[:], rsum[:])
            attnrow = smallp.tile([1, 16], f32, tag="attnrow")
            arview = attnrow[:].rearrange("o (a b) -> o a b", a=4)
            for r in range(4):
                nc.vector.tensor_scalar(
                    out=arview[:, r, :], in0=erow[:, r, :],
                    scalar1=rinv[:, r:r + 1], scalar2=None, op0=MULT)
            attnb = smallp.tile([128, 16], f32, tag="attnb")
            nc.gpsimd.partition_broadcast(attnb[:], attnrow[:])

            idents = []
            for k in range(16):
                idk = identp.tile([128, 128], f16, tag=f"id_{k}")
                nc.vector.tensor_scalar(
                    out=idk[:], in0=ident_base[:],
                    scalar1=attnb[:, k:k + 1], scalar2=None, op0=MULT)
                idents.append(idk)

            # ---- pass 2: out_d = sum_g attn[d,g] f_g -----------------------
            for hc in range(nhc):
                for bb in range(nbb):
                    fg = []
                    for g in range(D):
                        t = ft2p.tile([128, bb_w], f16, tag=f"ft2_{g}")
                        nc.sync.dma_start(
                            t[:],
                            ftap[g, hc * 128:(hc + 1) * 128,
                                 bb * bb_w:(bb + 1) * bb_w])
                        fg.append(t)
                    for d2 in range(D):
                        # balance pass 2 between TensorE (identity matmuls)
                        # and VectorE (4x tensor_scalar + 2x adds) -- both
                        # engines produce ~same tile rate, halving the span.
                        on_dve = (d2 == 3) or (d2 == 2 and hc % 2 == 1)
                        if on_dve:
                            acc = ostp.tile([128, bb_w], f16, tag="ost_dve")
                            tmp = workp.tile([128, bb_w], f16, tag="p2tmp")
                            nc.vector.tensor_scalar(
                                out=acc[:], in0=fg[0][:],
                                scalar1=attnb[:, d2 * 4:d2 * 4 + 1],
                                scalar2=None, op0=MULT)
                            for g in range(1, D):
                                nc.vector.tensor_scalar(
                                    out=tmp[:], in0=fg[g][:],
                                    scalar1=attnb[:, d2 * 4 + g:d2 * 4 + g + 1],
                                    scalar2=None, op0=MULT)
                                nc.vector.tensor_tensor(
                                    out=acc[:], in0=acc[:], in1=tmp[:], op=ADD)
                            nc.sync.dma_start(
                                out_d[d2, hc * 128:(hc + 1) * 128,
                                      bb * bb_w:(bb + 1) * bb_w], acc[:])
                            continue
                        po = psump.tile([128, bb_w], f32, tag="pm")
                        for m in range(nmm):
                            sl = slice(m * mm_w, (m + 1) * mm_w)
                            for g in range(D):
                                nc.tensor.matmul(
                                    po[:, sl], lhsT=idents[d2 * 4 + g][:],
                                    rhs=fg[g][:, sl],
                                    start=(g == 0), stop=(g == D - 1),
                                    skip_group_check=True)
                        os_t = ostp.tile([128, bb_w], f16, tag="ost")
                        nc.scalar.copy(os_t[:], po[:])
                        nc.sync.dma_start(
                            out_d[d2, hc * 128:(hc + 1) * 128,
                                  bb * bb_w:(bb + 1) * bb_w], os_t[:])

    nc.compile()
    return nc


def _get_nc(bl):
    if bl not in _CACHE:
        _CACHE[bl] = _build_nc(bl)
    return _CACHE[bl]


def _host_prep(feats, weights, bl):
    """Shard + transpose + cast inputs for each core."""
    import ml_dtypes
    ncores = feats.shape[1] // bl
    wtT = np.transpose(weights, (0, 2, 1))                    # [D, H_in, H_out]
    wt8 = np.ascontiguousarray((wtT * 16.0).astype(ml_dtypes.float8_e4m3))
    ftT = np.transpose(feats, (0, 2, 1))                      # [D, H, B]
    ftT16 = ftT.astype(np.float16)
    ftT8 = ftT.astype(ml_dtypes.float8_e4m3)
    in_maps = []
    for c in range(ncores):
        sl = slice(c * bl, (c + 1) * bl)
        in_maps.append({
            "ft": np.ascontiguousarray(ftT16[:, :, sl]),
            "ft8": np.ascontiguousarray(ftT8[:, :, sl]),
            "wt8": wt8,
        })
    return in_maps


def _assemble(results, bl):
    ncores = len(results)
    out = np.empty((D, ncores * bl, H), dtype=np.float32)
    for c, res in enumerate(results):
        # res["out"]: [D, H, bl] fp16
        out[:, c * bl:(c + 1) * bl, :] = np.transpose(
            res["out"].astype(np.float32), (0, 2, 1))
    return out


def run(feats, weights, trace=False, bl=BL_FULL, **spmd_kwargs):
    from concourse import bass_utils
    nc = _get_nc(bl)
    in_maps = _host_prep(np.asarray(feats), np.asarray(weights), bl)
    res = bass_utils.run_bass_kernel_spmd(
        nc, in_maps, core_ids=list(range(NCORES)), trace=trace, **spmd_kwargs)
    return _assemble(res.results, bl), res


def kernel(feats, weights):
    out, _ = run(np.asarray(feats), np.asarray(weights))
    return out
